# revision 1
# baseline (speedup 1.0000x reference)
"""AcceptRejectPooling2D on 8 Trainium2 NeuronCores.

Reference semantics (per 2x2 window, stride 2, NHWC):
    r  = relu(x)
    s  = sum(r); ss = sum(r*r)
    out = ss / s   if s > 0 else 0

Sharding: pure data parallel over batch (64 -> 8 per core). Each core
processes x_local [8, 64, 64, 256] -> y_local [8, 32, 32, 256].

Layout per core: rows (b, h) of length W*C = 16384 floats. Output row
p = (b, ho) needs input rows 2p (even h) and 2p+1 (odd h). 256 output
rows = 2 partition groups of 128. Row chunks of F floats stream through
SBUF; within a chunk the w-pair reduction is a strided tensor_add.
"""

import sys

if "/opt/trn_rl_repo" not in sys.path:
    sys.path.insert(0, "/opt/trn_rl_repo")

import numpy as np

_B, _H, _W, _C = 8, 64, 64, 256  # per-core shard
_HO, _WO = _H // 2, _W // 2
_NP = 128                         # SBUF partitions
_F = 2048                         # floats per row chunk (8 w * 256 c)
_FO = _F // 2
_NG = (_B * _HO) // _NP           # partition groups (2)
_NK = (_W * _C) // _F             # chunks per row (8)
_EPS = 1e-30

_CACHE = {}


def _build_nc():
    import concourse.bacc as bacc
    import concourse.tile as tile
    from concourse import mybir

    nc = bacc.Bacc("TRN2", target_bir_lowering=False, debug=False, num_devices=8)
    f32 = mybir.dt.float32
    x = nc.dram_tensor("x", [_B, _H, _W, _C], f32, kind="ExternalInput")
    y = nc.dram_tensor("y", [_B, _HO, _WO, _C], f32, kind="ExternalOutput")

    # [2, 256, 16384]: xv[par, (b, ho), (w, c)] with par = h % 2
    xv = x.ap().rearrange("b (hh par) w c -> par (b hh) (w c)", par=2)
    # [256, 8192]
    yv = y.ap().rearrange("b i j c -> (b i) (j c)")

    relu = mybir.ActivationFunctionType.Relu
    square = mybir.ActivationFunctionType.Square
    add = mybir.AluOpType.add

    def wpairs(t, n):
        # [128, n] -> even/odd w views [128, n//512, 256]
        v = t[:].rearrange("p (w par c) -> p w par c", par=2, c=_C)
        return v[:, :, 0, :], v[:, :, 1, :]

    def wflat(t):
        return t[:].rearrange("p (w c) -> p w c", c=_C)

    with tile.TileContext(nc) as tc:
        with (
            tc.tile_pool(name="io", bufs=3) as io,
            tc.tile_pool(name="tmp", bufs=2) as tmp,
        ):
            for g in range(_NG):
                for k in range(_NK):
                    E = io.tile([_NP, _F], f32, tag="E")
                    O = io.tile([_NP, _F], f32, tag="O")
                    src = xv[:, g * _NP:(g + 1) * _NP, k * _F:(k + 1) * _F]
                    nc.sync.dma_start(E[:], src[0])
                    nc.sync.dma_start(O[:], src[1])

                    rE = tmp.tile([_NP, _F], f32, tag="rE")
                    rO = tmp.tile([_NP, _F], f32, tag="rO")
                    qE = tmp.tile([_NP, _F], f32, tag="qE")
                    qO = tmp.tile([_NP, _F], f32, tag="qO")
                    nc.scalar.activation(rE[:], E[:], relu)
                    nc.scalar.activation(rO[:], O[:], relu)
                    nc.scalar.activation(qE[:], rE[:], square)
                    nc.scalar.activation(qO[:], rO[:], square)

                    sE = tmp.tile([_NP, _FO], f32, tag="sE")
                    sO = tmp.tile([_NP, _FO], f32, tag="sO")
                    s = tmp.tile([_NP, _FO], f32, tag="s")
                    ssE = tmp.tile([_NP, _FO], f32, tag="ssE")
                    ssO = tmp.tile([_NP, _FO], f32, tag="ssO")
                    ss = tmp.tile([_NP, _FO], f32, tag="ss")
                    t = tmp.tile([_NP, _FO], f32, tag="t")
                    o = tmp.tile([_NP, _FO], f32, tag="o")

                    rEe, rEo = wpairs(rE, _F)
                    rOe, rOo = wpairs(rO, _F)
                    qEe, qEo = wpairs(qE, _F)
                    qOe, qOo = wpairs(qO, _F)

                    nc.vector.tensor_add(wflat(sE), rEe, rEo)
                    nc.vector.tensor_add(wflat(sO), rOe, rOo)
                    # s = (sE + eps) + sO ; eps guards 1/0 for all-zero windows
                    nc.vector.scalar_tensor_tensor(
                        s[:], sE[:], _EPS, sO[:], op0=add, op1=add
                    )
                    nc.vector.tensor_add(wflat(ssE), qEe, qEo)
                    nc.vector.tensor_add(wflat(ssO), qOe, qOo)
                    nc.vector.tensor_add(ss[:], ssE[:], ssO[:])

                    nc.vector.reciprocal_approx_fast(t[:], s[:])
                    nc.vector.tensor_mul(o[:], ss[:], t[:])

                    nc.sync.dma_start(
                        yv[g * _NP:(g + 1) * _NP, k * _FO:(k + 1) * _FO], o[:]
                    )

    nc.compile()
    return nc


def _get_nc():
    if "nc" not in _CACHE:
        _CACHE["nc"] = _build_nc()
    return _CACHE["nc"]


def kernel(x: np.ndarray) -> np.ndarray:
    from concourse.bass_utils import run_bass_kernel_spmd

    nc = _get_nc()
    x = np.ascontiguousarray(np.asarray(x, dtype=np.float32))
    shards = np.split(x, 8, axis=0)
    in_maps = [{"x": s} for s in shards]
    res = run_bass_kernel_spmd(nc, in_maps, list(range(8)))
    return np.concatenate([res.results[i]["y"] for i in range(8)], axis=0)
